# revision 13
# baseline (speedup 1.0000x reference)
"""Trainium2 Bass kernel for nn_Embedding_loss (masked per-instance embedding loss).

Math: for each instance k with class c_k, over the (H,W) plane:
    cnt_k = sum(mask_k), s1_k = sum(emb[c_k] * mask_k), s2_k = sum(emb[c_k]^2 * mask_k)
With m1 = emb * mask and mask in {0,1}:  s2_k = sum(m1^2).
Per-instance means/variances plus the tiny O(K^2) pairwise hinge term are
assembled on the host from the (s1, s2, cnt) triples.

Sharding: perfectly balanced over 8 cores — 12 full instances per core plus
one HALF instance (instances 96..99 are split by pixel range across core
pairs), so every core owns exactly 12.5 instance-equivalents. The host
gathers each instance's class plane and mask as fp8 (0/1 exact for masks;
fp8 quantization of the embeddings moves the final loss by ~2e-5 relative),
laid out partition-major and column-flattened, and counts mask bits host-side
while staging.

Per-instance device pipeline (engines run in parallel, Tile double-buffers):
    VectorE: scalar_tensor_tensor m1 = plane*mask (fp8 reads), accum_out = s1
    ScalarE: Square(m1), accum_out = s2
The half slot runs first (small DMA -> compute starts sooner); full slots
stream in pairs to halve DMA/semaphore traffic.
"""

import os

import numpy as np

import concourse.bass as bass
import concourse.tile as tile
from concourse import mybir
from concourse.bass_utils import run_bass_kernel_spmd

N_CORES = 8
C, H, W = 80, 512, 512
K = 100
P = 128  # SBUF partitions
F = (H * W) // P  # free-dim elements per partition (2048)
NF = 12  # full instance slots per core
HALF = F // 2  # half-instance columns
COLS = HALF + NF * F  # flattened per-core column count (25600)
NSLOT = NF + 1  # stats slots (half first)

_NC_CACHE = None
LAST_RESULT = None  # BassKernelResults of the most recent run (for test harness)


def _split_sync(nc, max_w=1, max_u=1):
    """Walrus in this env accepts at most one sync wait/update per instruction;
    Tile's kernel-tail drain aggregates several. Split extras onto NoOps on the
    same engine (sequential waits on one queue are an AND, so semantics hold)."""
    ctr = 0
    for f in nc.m.functions:
        for bb in f.blocks:
            new = []
            for inst in bb.instructions:
                si = getattr(inst, "sync_info", None)
                waits = list(si.on_wait) if si is not None and si.on_wait else []
                updates = (
                    list(si.on_update) if si is not None and si.on_update else []
                )
                pre, post = [], []
                if len(waits) > max_w:
                    extra, keep = waits[:-max_w], waits[-max_w:]
                    si.on_wait = keep
                    for w in extra:
                        ctr += 1
                        nop = mybir.InstNoOp(name=f"syncsplit-w-{ctr}", ins=[], outs=[])
                        nop.engine = inst.engine
                        nop.sync_info = mybir.SyncInfo(on_wait=[w], on_update=[])
                        pre.append(nop)
                if len(updates) > max_u:
                    keep_u, extra_u = updates[:max_u], updates[max_u:]
                    si.on_update = keep_u
                    for u in extra_u:
                        ctr += 1
                        nop = mybir.InstNoOp(name=f"syncsplit-u-{ctr}", ins=[], outs=[])
                        nop.engine = inst.engine
                        nop.sync_info = mybir.SyncInfo(on_wait=[], on_update=[u])
                        post.append(nop)
                new.extend(pre)
                new.append(inst)
                new.extend(post)
            bb.instructions = new


def _emit_slot(nc, st, work, e_ap, m_ap, width, slot):
    """One instance-slot: fused multiply+s1 on VectorE, square+s2 on ScalarE."""
    m1 = work.tile([P, width], mybir.dt.float16, tag=f"m1_{width}")
    nc.vector.scalar_tensor_tensor(
        out=m1,
        in0=e_ap,
        scalar=1.0,
        in1=m_ap,
        op0=mybir.AluOpType.mult,
        op1=mybir.AluOpType.mult,
        accum_out=st[:, slot : slot + 1],
    )
    junk = work.tile([P, width], mybir.dt.float16, tag=f"junk_{width}")
    nc.scalar.activation(
        out=junk,
        in_=m1,
        func=mybir.ActivationFunctionType.Square,
        accum_out=st[:, NSLOT + slot : NSLOT + slot + 1],
    )


def _build_program():
    """One SPMD Bass program: half slot first, then 6 pairs of full slots."""
    global _NC_CACHE
    if _NC_CACHE is not None:
        return _NC_CACHE

    nc = bass.Bass()
    planes = nc.declare_dram_parameter(
        "planes", [P, COLS], mybir.dt.float8e4, isOutput=False
    )
    masks = nc.declare_dram_parameter(
        "masks", [P, COLS], mybir.dt.float8e4, isOutput=False
    )
    # stats columns: [0:NSLOT) = s1 partials, [NSLOT:2*NSLOT) = s2 partials
    stats = nc.declare_dram_parameter(
        "stats", [P, 2 * NSLOT], mybir.dt.float32, isOutput=True
    )

    with tile.TileContext(nc) as tc:
        with (
            tc.tile_pool(name="io", bufs=3) as io,
            tc.tile_pool(name="work", bufs=3) as work,
            tc.tile_pool(name="statp", bufs=1) as statp,
        ):
            st = statp.tile([P, 2 * NSLOT], mybir.dt.float32)

            # half slot first: small DMA lets compute start early
            eh = io.tile([P, HALF], mybir.dt.float8e4, tag="eh")
            mh = io.tile([P, HALF], mybir.dt.float8e4, tag="mh")
            nc.sync.dma_start(out=eh, in_=planes[:, 0:HALF])
            nc.sync.dma_start(out=mh, in_=masks[:, 0:HALF])
            _emit_slot(nc, st, work, eh, mh, HALF, 0)

            # 6 pairs of full slots
            for g in range(NF // 2):
                base = HALF + 2 * g * F
                eg = io.tile([P, 2 * F], mybir.dt.float8e4, tag="e")
                mg = io.tile([P, 2 * F], mybir.dt.float8e4, tag="m")
                nc.sync.dma_start(out=eg, in_=planes[:, base : base + 2 * F])
                nc.sync.dma_start(out=mg, in_=masks[:, base : base + 2 * F])
                for i in range(2):
                    _emit_slot(
                        nc,
                        st,
                        work,
                        eg[:, i * F : (i + 1) * F],
                        mg[:, i * F : (i + 1) * F],
                        F,
                        1 + 2 * g + i,
                    )

            nc.sync.dma_start(out=stats[:, :], in_=st)

    _NC_CACHE = nc
    return nc


def _enable_jax_compile_cache():
    try:
        import jax

        jax.config.update("jax_compilation_cache_dir", "/tmp/jax_neff_cache")
        jax.config.update("jax_persistent_cache_min_entry_size_bytes", -1)
        jax.config.update("jax_persistent_cache_min_compile_time_secs", 0.0)
    except Exception:
        pass
    # NEFF disk cache keyed on BIR bytes (deterministic serialization):
    # skip walrus recompiles across processes.
    try:
        import hashlib
        import shutil

        from concourse import bass2jax

        orig = bass2jax.compile_bir_kernel
        if getattr(orig, "_neff_cache_wrapped", False):
            return

        def cached_compile(bir_json, tmpdir, neff_name="file.neff"):
            h = hashlib.sha256(
                bir_json if isinstance(bir_json, bytes) else bir_json.encode()
            ).hexdigest()
            cpath = f"/tmp/neff_cache/{h}.neff"
            if os.path.exists(cpath):
                dst = os.path.join(tmpdir, neff_name)
                shutil.copy(cpath, dst)
                return dst
            out = orig(bir_json, tmpdir, neff_name=neff_name)
            os.makedirs("/tmp/neff_cache", exist_ok=True)
            shutil.copy(out, cpath)
            return out

        cached_compile._neff_cache_wrapped = True
        bass2jax.compile_bir_kernel = cached_compile
    except Exception:
        pass


def kernel(pred_emb, gt_objmask, gt_classes):
    global LAST_RESULT
    pred_emb = np.asarray(pred_emb)
    gt_objmask = np.asarray(gt_objmask)
    cls = np.clip(np.asarray(gt_classes).astype(np.int64), 0, C - 1)
    k = gt_objmask.shape[0]
    # layout below assumes exactly K=100 instances; zero-pad if fewer
    if k < K:
        gt_objmask = np.concatenate(
            [gt_objmask, np.zeros((K - k, H, W), dtype=gt_objmask.dtype)]
        )
        cls = np.concatenate([cls, np.zeros(K - k, dtype=np.int64)])
    assert gt_objmask.shape[0] == K, f"expected <= {K} instances, got {k}"

    _enable_jax_compile_cache()
    nc = _build_program()
    if not getattr(nc, "_sync_split_done", False):
        _split_sync(nc)  # CoreSim can't execute the bare NoOps; HW path only
        nc._sync_split_done = True

    f8 = mybir.dt.np(mybir.dt.float8e4)
    emb8 = pred_emb.astype(f8).reshape(C, P, F)
    one_f8 = np.ones((), dtype=f8).view(np.uint8)  # bit pattern of fp8 1.0
    mask8 = (gt_objmask.astype(np.uint8) * one_f8).view(f8).reshape(K, P, F)
    cnt = np.count_nonzero(gt_objmask.reshape(K, -1), axis=1).astype(np.float64)

    in_maps = []
    for c in range(N_CORES):
        pl = np.empty((P, COLS), dtype=f8)
        mk = np.empty((P, COLS), dtype=f8)
        # half slot: instance 96 + c//2, column half c%2
        hidx, part = K - 4 + c // 2, c % 2
        sl = slice(part * HALF, (part + 1) * HALF)
        pl[:, 0:HALF] = emb8[cls[hidx]][:, sl]
        mk[:, 0:HALF] = mask8[hidx][:, sl]
        # 12 full slots: instances 12c .. 12c+11
        lo = NF * c
        pl[:, HALF:] = emb8[cls[lo : lo + NF]].transpose(1, 0, 2).reshape(P, NF * F)
        mk[:, HALF:] = mask8[lo : lo + NF].transpose(1, 0, 2).reshape(P, NF * F)
        in_maps.append({"planes": pl, "masks": mk})

    core_ids = list(range(N_CORES))
    trace = bool(os.environ.get("KERNEL_TRACE"))
    res = run_bass_kernel_spmd(
        nc,
        in_maps,
        core_ids,
        trace=trace,
        trace_cores=core_ids if trace else None,
    )
    LAST_RESULT = res

    s1 = np.zeros(K, dtype=np.float64)
    s2 = np.zeros(K, dtype=np.float64)
    for c in range(N_CORES):
        stats = res.results[c]["stats"].astype(np.float64)  # (P, 2*NSLOT)
        ssum = stats.sum(axis=0)  # (2*NSLOT,)
        lo = NF * c
        s1[lo : lo + NF] += ssum[1:NSLOT]
        s2[lo : lo + NF] += ssum[NSLOT + 1 : 2 * NSLOT]
        hidx = K - 4 + c // 2
        s1[hidx] += ssum[0]
        s2[hidx] += ssum[NSLOT]

    # restrict to the caller's k instances (padding entries drop out)
    s1, s2, cnt, cls = s1[:k], s2[:k], cnt[:k], cls[:k]
    has = cnt > 0
    safe = np.where(has, cnt, 1.0)
    mean = np.where(has, s1 / safe, 0.0)
    var = np.where(has, s2 / safe - mean * mean, 0.0)

    same = cls[:, None] == cls[None, :]
    upper = np.triu(np.ones((k, k), dtype=bool), 1)
    diff2 = (mean[:, None] - mean[None, :]) ** 2
    hinge = np.maximum(1.0 - diff2, 0.0)
    loss_inter = np.sum(np.where(same & upper, hinge, 0.0))
    loss_reg = np.mean(mean * mean)
    loss_intra = np.mean(var)
    loss = 1.0 * loss_inter + 1.0 * loss_reg + 1.0 * loss_intra
    return np.array([loss], dtype=np.float32)
